# revision 1
# baseline (speedup 1.0000x reference)
"""Multi-head attention (B=2, S=2048, D=2048, H=16) on 8 trn2 NeuronCores.

Tensor-parallel over heads: core c owns heads [2c, 2c+1].
Per core:
  - Q/K/V projections as compensated-fp8 DoubleRow matmuls (x and W split
    into fp8 hi+lo host-side; 3-term product hi*hi + hi*lo + lo*hi ~= bf16
    accuracy at 0.75x the model cost of bf16)
  - attention core in fp16 (scores, softmax weights, AV)
  - softmax denominator via ones-matmul on DVE-accumulated fp16 partial sums
  - output projection compensated-fp8 DoubleRow (merged split hi/lo on-chip)
  - output partial [B, 16, 128, S] fp16, evacuated on ACT/DVE
Host: fp8 hi/lo packing of x/W, final sum of 8 partials + bo + Wo@bv.
"""

import numpy as np

try:
    import concourse.bass as bass  # noqa: F401
except ImportError:  # pragma: no cover - fresh grading dir
    import sys

    sys.path.insert(0, "/opt/trn_rl_repo")

import ml_dtypes

import concourse.bacc as bacc
import concourse.mybir as mybir
import concourse.tile as tile
from concourse.bass_utils import run_bass_kernel_spmd

import os

B, S, D, H = 2, 2048, 2048, 16
HD = D // H  # 128
N_CORES = 8
HPC = H // N_CORES  # heads per core = 2
CD = HPC * HD  # per-core projection dims = 256
TOK = B * S  # 4096

F32 = mybir.dt.float32
F16 = mybir.dt.float16
F8 = mybir.dt.float8e4
DR = mybir.MatmulPerfMode.DoubleRow
Act = mybir.ActivationFunctionType

TT = 512  # token tile
KC = D // 128  # contraction chunks = 16
NB = S // 128  # key blocks per batch = 16
NQ = S // TT  # q tiles per batch = 4
NT = S // TT  # token tiles per batch = 4
NDB = D // 128  # output dim blocks = 16
SCALE = 1.0 / float(np.sqrt(HD))

SX = 32.0  # x fp8 scale (e4m3 max finite = 240)
SW = 1024.0  # weight fp8 scale
SM = 32.0  # merged fp8 scale
DEQ = 1.0 / (SX * SW)  # proj psum dequant
DEQ_O = 1.0 / (SM * SW)  # out psum dequant


def build_program():
    nc = bacc.Bacc("TRN2", target_bir_lowering=False, debug=False, num_devices=N_CORES)

    xhl = nc.dram_tensor("xhl", [128, 2, KC, TOK], F8, kind="ExternalInput").ap()
    wq = nc.dram_tensor("wq", [128, 2, KC, CD], F8, kind="ExternalInput").ap()
    wk = nc.dram_tensor("wk", [128, 2, KC, CD], F8, kind="ExternalInput").ap()
    wv = nc.dram_tensor("wv", [128, 2, KC, CD], F8, kind="ExternalInput").ap()
    wo = nc.dram_tensor("wo", [128, 2, HPC, D], F8, kind="ExternalInput").ap()
    bq = nc.dram_tensor("bq", [CD], F32, kind="ExternalInput").ap()
    bk = nc.dram_tensor("bk", [CD], F32, kind="ExternalInput").ap()
    out = nc.dram_tensor("out", [B, NDB, 128, S], F16, kind="ExternalOutput").ap()

    with tile.TileContext(nc) as tc:
        _build_tile(nc, tc, xhl, wq, wk, wv, wo, bq, bk, out)

    nc.compile()
    return nc


def _build_tile(nc, tc, xhl, wq, wk, wv, wo, bq, bk, out):
    import contextlib

    ctx = contextlib.ExitStack()
    with ctx:
        const = ctx.enter_context(tc.tile_pool(name="const", bufs=1))
        xpool = ctx.enter_context(tc.tile_pool(name="x", bufs=2))
        qkv = ctx.enter_context(tc.tile_pool(name="qkv", bufs=2))
        mt_p = ctx.enter_context(tc.tile_pool(name="mt", bufs=2))
        est_p = ctx.enter_context(tc.tile_pool(name="est", bufs=8))
        small = ctx.enter_context(tc.tile_pool(name="small", bufs=4))
        outp = ctx.enter_context(tc.tile_pool(name="outp", bufs=6))
        # PSUM (8 banks x 2KB/partition): mm 2x2 + acc 2x1 + o 2x1 = 8
        ps_mm = ctx.enter_context(tc.tile_pool(name="ps_mm", bufs=2, space="PSUM"))
        ps_acc = ctx.enter_context(tc.tile_pool(name="ps_acc", bufs=2, space="PSUM"))
        ps_o = ctx.enter_context(tc.tile_pool(name="ps_o", bufs=2, space="PSUM"))

        # ---- resident constants, staged so first matmuls start ASAP ----
        wq_sb = const.tile([128, 2, KC, CD], F8, tag="wq")
        wk_sb = const.tile([128, 2, KC, CD], F8, tag="wk")
        wv_sb = const.tile([128, 2, KC, CD], F8, tag="wv")
        wo_sb = const.tile([128, 2, HPC, D], F8, tag="wo")

        xt0 = xpool.tile([128, 2, KC, TT], F8, tag="xt", name="xt0")
        # hi slots first, in fine chunk pieces so the first matmuls start ASAP
        for lo_, hi_ in ((0, 2), (2, 4), (4, 8), (8, 16)):
            nc.sync.dma_start(xt0[:, 0, lo_:hi_, :], xhl[:, 0, lo_:hi_, 0:TT])
            nc.sync.dma_start(wq_sb[:, 1, lo_:hi_, :], wq[:, 1, lo_:hi_, :])
        bq_sb = const.tile([128, HPC], F32, tag="bq")
        nc.sync.dma_start(bq_sb[:], bq.rearrange("(h p) -> p h", p=128))
        bk_sb = const.tile([128, HPC], F32, tag="bk")
        nc.sync.dma_start(bk_sb[:], bk.rearrange("(h p) -> p h", p=128))
        for lo_, hi_ in ((0, 4), (4, 10), (10, 16)):
            nc.sync.dma_start(xt0[:, 1, lo_:hi_, :], xhl[:, 1, lo_:hi_, 0:TT])
            nc.sync.dma_start(wq_sb[:, 0, lo_:hi_, :], wq[:, 0, lo_:hi_, :])
        for lo_, hi_ in ((0, 8), (8, 16)):
            nc.sync.dma_start(wk_sb[:, 1, lo_:hi_, :], wk[:, 1, lo_:hi_, :])
        nc.sync.dma_start(wk_sb[:, 0, :, :], wk[:, 0, :, :])
        nc.sync.dma_start(wv_sb[:], wv)

        ones16 = const.tile([128, 128], F16, tag="ones")
        nc.vector.memset(ones16[:], 1.0 / SM)

        pending_p3 = []  # deferred filler-work emitters (p1 tiles, out-proj)
        states = {}
        debt = [0.0]

        def pump(ns):
            """Advance queued filler generators by ~ns of PE work."""
            debt[0] += ns
            while debt[0] > 0 and pending_p3:
                try:
                    cost = next(pending_p3[0])
                except StopIteration:
                    pending_p3.pop(0)
                    continue
                debt[0] -= cost
            if not pending_p3 and debt[0] > 0:
                debt[0] = 0.0

        def get_state(b):
            if b not in states:
                states[b] = {
                    "QT": qkv.tile([128, HPC, S], F16, tag="QT", name=f"QT{b}"),
                    "KT": qkv.tile([128, HPC, S], F16, tag="KT", name=f"KT{b}"),
                    "V": qkv.tile([128, NB, CD], F16, tag="V", name=f"V{b}"),
                }
            return states[b]

        def p1_steps(b, t0, ntb):
            """Project a tile of ntb 128-token blocks starting at block t0."""
            st = get_state(b)
            QT, KT, V = st["QT"], st["KT"], st["V"]
            w = ntb * 128
            off = b * S + t0 * 128
            if b == 0 and t0 == 0:
                xt = xt0
            else:
                xt = xpool.tile([128, 2, KC, w], F8, tag="xt", name=f"xt{b}_{t0}")
                nc.sync.dma_start(xt[:, 0, :, :], xhl[:, 0, :, off : off + w])
                nc.sync.dma_start(xt[:, 1, :, :], xhl[:, 1, :, off : off + w])
            tsl = slice(t0 * 128, t0 * 128 + w)
            cold = b == 0 and t0 == 0
            for w_sb, bias_sb, dst in ((wq_sb, bq_sb, QT), (wk_sb, bk_sb, KT)):
                if cold:
                    # DMA-bound startup: run both blocks' hi passes first so
                    # PE has work while the lo slots stream in.
                    pps = [
                        ps_mm.tile([128, w], F32, tag="mm", name=f"cold_{id(w_sb)}_{i}")
                        for i in range(HPC)
                    ]
                    for c2 in range(KC // 2):
                        for blk in range(HPC):
                            msl = slice(blk * HD, (blk + 1) * HD)
                            nc.tensor.matmul(
                                pps[blk][:],
                                w_sb[:, 1, 2 * c2 : 2 * c2 + 2, msl],
                                xt[:, 0, 2 * c2 : 2 * c2 + 2, :],
                                start=(c2 == 0),
                                stop=False,
                                perf_mode=DR,
                            )
                    for c in range(KC):
                        for blk in range(HPC):
                            msl = slice(blk * HD, (blk + 1) * HD)
                            nc.tensor.matmul(
                                pps[blk][:],
                                w_sb[:, :, c, msl],
                                xt[:, :, c, :],
                                start=False,
                                stop=(c == KC - 1),
                                perf_mode=DR,
                            )
                    for blk in range(HPC):
                        nc.scalar.activation(
                            dst[:, blk, tsl],
                            pps[blk][:],
                            Act.Identity,
                            scale=DEQ,
                            bias=bias_sb[:, blk : blk + 1],
                        )
                        yield 640 * ntb
                    continue
                for blk in range(HPC):
                    msl = slice(blk * HD, (blk + 1) * HD)
                    p_ps = ps_mm.tile([128, w], F32, tag="mm")
                    for c2 in range(KC // 2):
                        nc.tensor.matmul(
                            p_ps[:],
                            w_sb[:, 1, 2 * c2 : 2 * c2 + 2, msl],
                            xt[:, 0, 2 * c2 : 2 * c2 + 2, :],
                            start=(c2 == 0),
                            stop=False,
                            perf_mode=DR,
                        )
                    for c in range(KC):
                        nc.tensor.matmul(
                            p_ps[:],
                            w_sb[:, :, c, msl],
                            xt[:, :, c, :],
                            start=False,
                            stop=(c == KC - 1),
                            perf_mode=DR,
                        )
                    nc.scalar.activation(
                        dst[:, blk, tsl],
                        p_ps[:],
                        Act.Identity,
                        scale=DEQ,
                        bias=bias_sb[:, blk : blk + 1],
                    )
                    yield 640 * ntb
            for tb in range(ntb):
                v_ps = ps_o.tile([128, CD], F32, tag="o")
                bsl = slice(tb * 128, (tb + 1) * 128)
                for c2 in range(KC // 2):
                    nc.tensor.matmul(
                        v_ps[:],
                        xt[:, 0, 2 * c2 : 2 * c2 + 2, bsl],
                        wv_sb[:, 1, 2 * c2 : 2 * c2 + 2, :],
                        start=(c2 == 0),
                        stop=False,
                        perf_mode=DR,
                    )
                for c in range(KC):
                    nc.tensor.matmul(
                        v_ps[:],
                        xt[:, :, c, bsl],
                        wv_sb[:, :, c, :],
                        start=False,
                        stop=(c == KC - 1),
                        perf_mode=DR,
                    )
                nc.scalar.mul(V[:, t0 + tb, :], v_ps[:], DEQ)
                yield 1280

        def attention_group(b, qt, evac_acts, kp_pump, tail_pump, pump_delay=0):
            QT, KT, V = (get_state(b)[k] for k in ("QT", "KT", "V"))
            if True:
                qsl = slice(qt * TT, (qt + 1) * TT)
                mt8 = mt_p.tile([128, 2, HPC, TT], F8, tag="MT")
                for h in range(HPC):
                    attn_ps = ps_acc.tile([128, TT], F32, tag="acc")
                    dacc = small.tile([128, 2, TT], F16, tag="dacc")
                    ests = [None] * (NB // 2)

                    def av_pair(kp):
                        for j in range(2):
                            kb = 2 * kp + j
                            nc.tensor.matmul(
                                attn_ps[:],
                                V[:, kb, h * HD : (h + 1) * HD],
                                ests[kp][:, j, :],
                                start=(kb == 0),
                                stop=(kb == NB - 1),
                            )

                    for kp in range(NB // 2):
                        st_ps = ps_mm.tile([128, 2, TT], F32, tag="mm")
                        for j in range(2):
                            kb = 2 * kp + j
                            nc.tensor.matmul(
                                st_ps[:, j, :],
                                KT[:, h, kb * 128 : (kb + 1) * 128],
                                QT[:, h, qsl],
                                start=True,
                                stop=True,
                            )
                        est = est_p.tile([128, 2, TT], F16, tag="est")
                        nc.scalar.activation(est[:], st_ps[:], Act.Exp, scale=SCALE)
                        ests[kp] = est
                        if kp == 0:
                            nc.vector.tensor_copy(dacc[:], est[:])
                        else:
                            nc.vector.tensor_add(dacc[:], dacc[:], est[:])
                        if kp > 1:
                            av_pair(kp - 2)
                        if kp >= pump_delay:
                            pump(kp_pump)
                    pump(tail_pump)
                    av_pair(NB // 2 - 2)
                    av_pair(NB // 2 - 1)
                    pump(600)
                    dn_ps = ps_o.tile([128, TT], F32, tag="o")
                    nc.tensor.matmul(
                        dn_ps[:], ones16[:], dacc[:, 0, :], start=True, stop=False
                    )
                    nc.tensor.matmul(
                        dn_ps[:], ones16[:], dacc[:, 1, :], start=False, stop=True
                    )
                    recip = small.tile([128, TT], F32, tag="recip")
                    nc.vector.reciprocal(recip[:], dn_ps[:])
                    tmp16 = small.tile([128, TT], F16, tag="tmp")
                    nc.vector.tensor_mul(tmp16[:], attn_ps[:], recip[:])
                    nc.vector.tensor_copy(mt8[:, 0, h, :], tmp16[:])
                    nc.vector.scalar_tensor_tensor(
                        mt8[:, 1, h, :],
                        mt8[:, 0, h, :],
                        -1.0,
                        tmp16[:],
                        mybir.AluOpType.mult,
                        mybir.AluOpType.add,
                    )

                pending_p3.append(
                    _p3_steps(nc, ps_o, outp, wo_sb, mt8, out, b, qsl, evac_acts)
                )

        # schedule: p1(b0) back-to-back; p1(b1) + oproj pumped inside the
        # attention(b0) kp loops; attention(b1) with oproj pumped inside.
        for t in range(NT):
            for _ in p1_steps(0, t * 4, 4):
                pass
            if t == 0:
                nc.sync.dma_start(wo_sb[:], wo)
        _e = os.environ
        ea, eb = int(_e.get("EVA", "8")), int(_e.get("EVB", "6"))
        kpa, kpb = int(_e.get("KPA", "1300")), int(_e.get("KPB", "1200"))
        tla, tlb = int(_e.get("TLA", "1600")), int(_e.get("TLB", "1600"))
        pdb = int(_e.get("PDB", "3"))
        eb3 = int(_e.get("EB3", "8"))
        p1_gens = set()
        for i in range(NQ):
            g = p1_steps(1, i * 4, 4)
            p1_gens.add(g)
            pending_p3.insert(0, g)
            attention_group(0, i, evac_acts=ea, kp_pump=kpa, tail_pump=tla)
        # all p1(b1) work must be emitted before attention(b1) reads QT1/KT1/V1;
        # out-projection generators may keep spreading into the b1 groups.
        for g in list(pending_p3):
            if g in p1_gens:
                for _ in g:
                    pass
                pending_p3.remove(g)
        for i in range(NQ):
            attention_group(
                1,
                i,
                evac_acts=(eb3 if i == NQ - 1 else eb),
                kp_pump=kpb,
                tail_pump=tlb,
                pump_delay=pdb,
            )
        while pending_p3:
            for _ in pending_p3.pop(0):
                pass


def _p3_steps(nc, ps_o, outp, wo_sb, mt8, out, b, qsl, evac_acts):
    """Generator: one out-projection dblk per next() — injected between
    attention matmul pairs to fill PE gaps. evac_acts of the 16 dblk
    evacuations go to ACT, the rest to DVE."""
    for dblk in range(NDB):
        o_ps = ps_o.tile([128, TT], F32, tag="o", name=f"o_ps{b}_{dblk}")
        dsl = slice(dblk * 128, (dblk + 1) * 128)
        nc.tensor.matmul(
            o_ps[:],
            wo_sb[:, 1, :, dsl],
            mt8[:, 0, :, :],
            start=True,
            stop=False,
            perf_mode=DR,
        )
        for h in range(HPC):
            nc.tensor.matmul(
                o_ps[:],
                wo_sb[:, :, h, dsl],
                mt8[:, :, h, :],
                start=False,
                stop=(h == HPC - 1),
                perf_mode=DR,
            )
        o16 = outp.tile([128, TT], F16, tag="o", name=f"o16_{b}_{dblk}")
        if (dblk * evac_acts) % NDB < evac_acts:
            nc.scalar.mul(o16[:], o_ps[:], DEQ_O)
        else:
            nc.vector.tensor_scalar_mul(o16[:], o_ps[:], DEQ_O)
        nc.sync.dma_start(out[b, dblk, :, qsl], o16[:])
        yield 320


_program = None


def _get_program():
    global _program
    if _program is None:
        _program = build_program()
    return _program


def _hi_lo(arr, scale):
    """fp8 hi/lo split of arr*scale (both stored at the same scale).

    mybir float8e4 is IEEE e4m3 (ml_dtypes.float8_e4m3): max finite 240,
    overflows to inf — clip defensively."""
    f8 = ml_dtypes.float8_e4m3
    s = np.clip((arr * scale).astype(np.float32), -224.0, 224.0)
    hi = s.astype(f8)
    lo = np.clip(s - hi.astype(np.float32), -224.0, 224.0).astype(f8)
    return hi, lo


def kernel(x, Wq, bq, Wk, bk, Wv, bv, Wo, bo):
    x = np.asarray(x, np.float32)
    Wq, Wk, Wv, Wo = (np.asarray(w, np.float32) for w in (Wq, Wk, Wv, Wo))
    bq, bk, bv, bo = (np.asarray(v, np.float32) for v in (bq, bk, bv, bo))
    f8 = ml_dtypes.float8_e4m3

    # x -> [128, 2, KC, TOK] fp8 hi/lo, chunk c = contraction rows 128c..
    xT = np.ascontiguousarray(x.reshape(TOK, D).T).reshape(KC, 128, TOK)
    xhi, xlo = _hi_lo(xT, SX)
    xhl = np.empty((128, 2, KC, TOK), f8)
    xhl[:, 0] = xhi.transpose(1, 0, 2)
    xhl[:, 1] = xlo.transpose(1, 0, 2)

    nc = _get_program()
    in_maps = []
    for c in range(N_CORES):
        sl = slice(c * CD, (c + 1) * CD)

        def w_pack(mat):  # mat [D, CD] -> [128, 2(lo,hi), KC, CD]
            m3 = np.ascontiguousarray(mat).reshape(KC, 128, -1)
            hi, lo = _hi_lo(m3, SW)
            o = np.empty((128, 2, KC, mat.shape[1]), f8)
            o[:, 0] = lo.transpose(1, 0, 2)
            o[:, 1] = hi.transpose(1, 0, 2)
            return o

        wo_mat = np.ascontiguousarray(Wo[:, sl].T).reshape(HPC, 128, D)
        wo_hi, wo_lo = _hi_lo(wo_mat, SW)
        wo_p = np.empty((128, 2, HPC, D), f8)
        wo_p[:, 0] = wo_lo.transpose(1, 0, 2)
        wo_p[:, 1] = wo_hi.transpose(1, 0, 2)

        in_maps.append(
            {
                "xhl": xhl,
                "wq": w_pack(Wq[sl, :].T),
                "wk": w_pack(Wk[sl, :].T),
                "wv": w_pack(Wv[sl, :].T),
                "wo": wo_p,
                "bq": np.ascontiguousarray(bq[sl]),
                "bk": np.ascontiguousarray(bk[sl]),
            }
        )

    res = run_bass_kernel_spmd(nc, in_maps, core_ids=list(range(N_CORES)))
    acc = np.zeros((B, NDB, 128, S), np.float32)
    for r in res.results:
        acc += np.asarray(r["out"], np.float32)
    # [B, NDB, 128, S] -> [B, S, D]
    merged = acc.transpose(0, 3, 1, 2).reshape(B, S, D)
    return merged + (bo + Wo @ bv)

